# revision 1
# baseline (speedup 1.0000x reference)
"""Trainium2 Bass kernel for nn_AF_2 (dense per-branch MLP gating).

Math (reference):
    s = t.sum(axis=1)                                  # (B, D)
    h = relu(BN1(einsum('nid,bd->bni', W1, s) + b1))   # (B, NB, I)
    y = BN2(einsum('ndi,bni->bnd', W2, h) + b2)        # (B, NB, D)
    out = (sigmoid(y) * t).sum(axis=1) * 3             # (B, D)

Strategy (v4):
  - 8-way data parallel over B (512 rows/core), zero collectives.
  - Host folds the (inference-mode) BatchNorms into W/b, pre-scales t by 3
    (compensated in W1) so the gating product is a plain multiply, and packs
    ALL device inputs into ONE bf16 DRAM blob (t | W1 | W2 | ident | b1 | b2)
    to minimize per-execution argument overhead (biases are expanded back to
    f32 in SBUF via a casting SWDGE DMA).
  - t is streamed twice (pass A: s accumulation, pass B: gating product)
    in 1MB chunks, alternating between BOTH HWDGE queues (SP + Act) in
    pass A; pass B chunks ride the SP queue while weights trickle in on
    the Pool SWDGE queue, paced by the branch pipeline.
  - s^T accumulated with TensorE identity-matmuls into PSUM (exact f32).
  - GEMM1/GEMM2 are per-branch bf16 matmuls (free dim 512) in PSUM f32.
  - BN1+relu fused on VectorE (dual-op tensor_scalar: add bias, max 0);
    BN2 bias fused into the ScalarE sigmoid (per-partition bias).
  - r^T += w^T (.) (3t)^T: bf16 tensor_tensor product on VectorE (2x mode);
    accumulation split between TensorE identity-matmuls (2 d-chunks) and
    GPSIMD adds (2 d-chunks).
  - Single packed output DMA; host unpacks/transposes.
"""

import os
import sys

import numpy as np

sys.path.insert(0, "/opt/trn_rl_repo")

import ml_dtypes

B, NB, DIM, R = 4096, 64, 512, 4
INTER = DIM // R  # 128
EPS = 1e-5
NCORES = 8
BS = B // NCORES  # 512 rows per core
NDC = DIM // 128  # 4 d-chunks
NG = 8            # branches per t DMA group
NGRP = NB // NG   # 8 groups
NSLOT = 6         # t ring-buffer slots

# blob column offsets (bf16 blob, [128, CB])
T_COLS = NDC * NB * BS          # 131072
W_COLS = NB * NDC * INTER       # 32768
OW1 = T_COLS
OW2 = OW1 + W_COLS
OID = OW2 + W_COLS
OB1 = OID + 128
OB2 = OB1 + NB
CB = OB2 + NB * NDC             # 197056

_CACHE = {}


def _build_nc():
    import concourse.bass as bass
    import concourse.mybir as mybir

    bf16 = mybir.dt.bfloat16
    f32 = mybir.dt.float32
    AF = mybir.ActivationFunctionType
    OP = mybir.AluOpType

    nc = bass.Bass("TRN2", debug=False, target_bir_lowering=False)

    blob_ext = nc.declare_dram_parameter("blob", [128, CB], bf16, isOutput=False)
    out_ext = nc.declare_dram_parameter("out", [128, NDC * BS], f32, isOutput=True)

    def tcol(g, dc):
        return dc * (NB * BS) + g * (NG * BS)

    R_PE = (0, 1)    # dc accumulated on PE (PSUM, exact)
    R_POOL = (2, 3)  # dc accumulated on GPSIMD (f32 adds in SBUF)

    from contextlib import ExitStack
    ctx = ExitStack()
    with ctx:
        s_w = ctx.enter_context(nc.semaphore("s_w"))
        s_out = ctx.enter_context(nc.semaphore("s_out"))
        s_slot = [ctx.enter_context(nc.semaphore(f"s_slot{i}")) for i in range(NSLOT)]
        s_pe = ctx.enter_context(nc.semaphore("s_pe"))
        s_act = ctx.enter_context(nc.semaphore("s_act"))
        s_dve = ctx.enter_context(nc.semaphore("s_dve"))
        s_pool = ctx.enter_context(nc.semaphore("s_pool"))
        ident_sb = ctx.enter_context(nc.sbuf_tensor("ident_sb", [128, 128], bf16))
        w1_sb = ctx.enter_context(nc.sbuf_tensor("w1_sb", [128, NB, NDC, INTER], bf16))
        w2_sb = ctx.enter_context(nc.sbuf_tensor("w2_sb", [INTER, NB, NDC, 128], bf16))
        b1_sb = ctx.enter_context(nc.sbuf_tensor("b1_sb", [INTER, NB], f32))
        b2_sb = ctx.enter_context(nc.sbuf_tensor("b2_sb", [128, NB, NDC], f32))
        tt_sb = ctx.enter_context(nc.sbuf_tensor("tt_sb", [128, NSLOT, NG, BS], bf16))
        s_sb = ctx.enter_context(nc.sbuf_tensor("s_sb", [128, NDC, BS], bf16))
        h_sb = ctx.enter_context(nc.sbuf_tensor("h_sb", [INTER, 2, BS], bf16))
        w_sb = ctx.enter_context(nc.sbuf_tensor("w_sb", [128, NDC, 2, BS], bf16))
        p_sb = ctx.enter_context(nc.sbuf_tensor("p_sb", [128, NDC, 2, BS], bf16))
        racc_sb = ctx.enter_context(nc.sbuf_tensor("racc_sb", [128, NDC, BS], f32))
        ps_s = [ctx.enter_context(nc.psum_tensor(f"ps_s{dc}", [128, BS], f32)) for dc in range(NDC)]
        ps_y = [ctx.enter_context(nc.psum_tensor(f"ps_y{i}", [128, BS], f32)) for i in range(2)]
        ps_h = [ctx.enter_context(nc.psum_tensor(f"ps_h{i}", [INTER, BS], f32)) for i in range(2)]
        y_bank = {0: ps_y[0], 1: ps_y[1], 2: ps_s[2], 3: ps_s[3]}
        r_bank = {0: ps_s[0], 1: ps_s[1]}

        # ---------- static schedule ----------
        n_tdma = 2 * NGRP * NDC
        tslot_done = [16 * (k // NSLOT + 1) for k in range(n_tdma)]

        def tk(phase, g, dc):
            return phase * NGRP * NDC + g * NDC + dc

        pe_groupA_done = [None] * (NGRP * NDC)
        h_ready = [None] * NB
        y_ready = [[None] * NDC for _ in range(NB)]
        racc_done_pe = [[None] * NDC for _ in range(NB)]
        relu_done = [None] * NB
        sig_done = [[None] * NDC for _ in range(NB)]
        scopy_done = [None] * NDC
        stt_done = [[None] * NDC for _ in range(NB)]
        pool_done = [[None] * NDC for _ in range(NB)]
        rcopy_done = [None] * NDC

        pe_i = 0
        for k in range(NGRP * NDC):
            pe_i += 1
            pe_groupA_done[k] = pe_i
        pe_i += 1
        h_ready[0] = pe_i
        for n in range(NB):
            if n + 1 < NB:
                pe_i += 1
                h_ready[n + 1] = pe_i
            for dc in range(NDC):
                pe_i += 1
                y_ready[n][dc] = pe_i
            if n > 0:
                for dc in R_PE:
                    pe_i += 1
                    racc_done_pe[n - 1][dc] = pe_i
        for dc in R_PE:
            pe_i += 1
            racc_done_pe[NB - 1][dc] = pe_i

        act_i = 0
        for n in range(NB):
            for dc in range(NDC):
                act_i += 1
                sig_done[n][dc] = act_i

        # DVE order: scopy x4, relu(0), then per n: [relu(n+1)], stt x4;
        # finally rcopy x2
        dve_i = 0
        for dc in range(NDC):
            dve_i += 1
            scopy_done[dc] = dve_i
        dve_i += 1
        relu_done[0] = dve_i
        for n in range(NB):
            if n + 1 < NB:
                dve_i += 1
                relu_done[n + 1] = dve_i
            for dc in range(NDC):
                dve_i += 1
                stt_done[n][dc] = dve_i
        for dc in R_PE:
            dve_i += 1
            rcopy_done[dc] = dve_i

        pool_i = 0
        for n in range(NB):
            for dc in R_POOL:
                pool_i += 1
                pool_done[n][dc] = pool_i

        # weight-group availability: s_w counts (ident, b1, b2, w1g0, w2g0,
        # w1g1, w2g1, then per-group pairs issued inside the pool r-loop)
        def wg_done(g):
            return 16 * (3 + 2 * (g + 1))

        # t-chunk DMA issue helpers (phase A chunks alternate SP/Act)
        def issue_phaseA(eng, parity):
            for k in range(NGRP * NDC):
                if k % 2 != parity:
                    continue
                g, dc = divmod(k, NDC)
                slot = k % NSLOT
                if k >= NSLOT:
                    eng.wait_ge(s_pe, pe_groupA_done[k - NSLOT])
                c0 = tcol(g, dc)
                eng.dma_start(
                    out=tt_sb[:, slot, :, :],
                    in_=blob_ext[:, c0:c0 + NG * BS],
                ).then_inc(s_slot[slot], 16)

        with nc.Block() as block:

            # ================= SP: phase-A (even chunks) + all phase-B t + out =================
            @block.sync
            def _(sp):
                issue_phaseA(sp, 0)
                for g in range(NGRP):
                    for dc in range(NDC):
                        if dc == 3 and g >= 1:
                            continue  # issued from the Act queue
                        k = tk(1, g, dc)
                        slot = k % NSLOT
                        pk = k - NSLOT
                        if pk < NGRP * NDC:
                            sp.wait_ge(s_pe, pe_groupA_done[pk])
                        else:
                            m = pk - NGRP * NDC
                            pg, pdc = divmod(m, NDC)
                            pn = pg * NG + NG - 1
                            sp.wait_ge(s_dve, stt_done[pn][pdc])
                        c0 = tcol(g, dc)
                        sp.dma_start(
                            out=tt_sb[:, slot, :, :],
                            in_=blob_ext[:, c0:c0 + NG * BS],
                        ).then_inc(s_slot[slot], 16)
                for dc in R_PE:
                    sp.wait_ge(s_dve, rcopy_done[dc])
                for dc in R_POOL:
                    sp.wait_ge(s_pool, pool_done[NB - 1][dc])
                sp.dma_start(out=out_ext[:, :], in_=racc_sb[:, :, :]).then_inc(s_out, 16)
                sp.wait_ge(s_out, 16)

            # ================= PE =================
            @block.tensor
            def _(pe):
                pe.wait_ge(s_w, 16 * 3)  # ident+biases
                for g in range(NGRP):
                    for dc in range(NDC):
                        k = tk(0, g, dc)
                        slot = k % NSLOT
                        pe.wait_ge(s_slot[slot], tslot_done[k])
                        for j in range(NG):
                            mm = pe.matmul(
                                ps_s[dc][:, :], lhsT=ident_sb[:, :],
                                rhs=tt_sb[:, slot, j, :],
                                start=(g == 0 and j == 0), stop=(g == NGRP - 1 and j == NG - 1),
                            )
                        mm.then_inc(s_pe, 1)
                # prologue: G1(0)
                pe.wait_ge(s_dve, scopy_done[NDC - 1])
                pe.wait_ge(s_w, wg_done(0))
                for dc in range(NDC):
                    mm = pe.matmul(
                        ps_h[0][:, :], lhsT=w1_sb[:, 0, dc, :], rhs=s_sb[:, dc, :],
                        start=(dc == 0), stop=(dc == NDC - 1),
                    )
                mm.then_inc(s_pe, 1)
                for n in range(NB):
                    # G1(n+1) hoisted: h always ready one branch ahead
                    if n + 1 < NB:
                        if (n + 1) % NG == 0:
                            pe.wait_ge(s_w, wg_done((n + 1) // NG))
                        for dc in range(NDC):
                            mm = pe.matmul(
                                ps_h[(n + 1) % 2][:, :], lhsT=w1_sb[:, n + 1, dc, :], rhs=s_sb[:, dc, :],
                                start=(dc == 0), stop=(dc == NDC - 1),
                            )
                        mm.then_inc(s_pe, 1)
                    pe.wait_ge(s_dve, relu_done[n])
                    for dc in range(NDC):
                        if n > 0:
                            pe.wait_ge(s_act, sig_done[n - 1][dc])
                        elif dc >= 2:
                            pe.wait_ge(s_dve, scopy_done[dc])
                        pe.matmul(
                            y_bank[dc][:, :], lhsT=w2_sb[:, n, dc, :], rhs=h_sb[:, n % 2, :],
                            start=True, stop=True,
                        ).then_inc(s_pe, 1)
                    if n > 0:
                        for dc in R_PE:
                            pe.wait_ge(s_dve, stt_done[n - 1][dc])
                            pe.matmul(
                                r_bank[dc][:, :], lhsT=ident_sb[:, :], rhs=p_sb[:, dc, (n - 1) % 2, :],
                                start=(n - 1 == 0), stop=False,
                            ).then_inc(s_pe, 1)
                for dc in R_PE:
                    pe.wait_ge(s_dve, stt_done[NB - 1][dc])
                    pe.matmul(
                        r_bank[dc][:, :], lhsT=ident_sb[:, :], rhs=p_sb[:, dc, (NB - 1) % 2, :],
                        start=False, stop=True,
                    ).then_inc(s_pe, 1)

            # ================= ACT: phase-A odd t chunks, then sigmoid =================
            @block.scalar
            def _(act):
                issue_phaseA(act, 1)
                act.wait_ge(s_w, 16 * 3)
                for n in range(NB):
                    if n >= 2:
                        act.wait_ge(s_dve, stt_done[n - 2][NDC - 1])
                    for dc in range(NDC):
                        act.wait_ge(s_pe, y_ready[n][dc])
                        act.activation(
                            w_sb[:, dc, n % 2, :], y_bank[dc][:, :], AF.Sigmoid,
                            bias=b2_sb[:, n, dc:dc + 1], scale=1.0,
                        ).then_inc(s_act, 1)
                    if (n + 1) % NG == 0 and 1 <= (n + 1) // NG < NGRP:
                        gq = (n + 1) // NG
                        k = tk(1, gq, 3)
                        pk = k - NSLOT
                        m = pk - NGRP * NDC
                        pg, pdc = divmod(m, NDC)
                        act.wait_ge(s_dve, stt_done[pg * NG + NG - 1][pdc])
                        c0 = tcol(gq, 3)
                        act.dma_start(
                            out=tt_sb[:, k % NSLOT, :, :],
                            in_=blob_ext[:, c0:c0 + NG * BS],
                        ).then_inc(s_slot[k % NSLOT], 16)

            # ================= DVE: s-copies, relu(+bias), w*t product, r-copies =================
            @block.vector
            def _(dve):
                dve.wait_ge(s_pe, pe_groupA_done[NGRP * NDC - 1])
                for dc in range(NDC):
                    dve.tensor_copy(s_sb[:, dc, :], ps_s[dc][:, :]).then_inc(s_dve, 1)
                dve.wait_ge(s_w, 16 * 3)
                dve.wait_ge(s_pe, h_ready[0])
                dve.tensor_scalar(
                    h_sb[:, 0, :], ps_h[0][:, :], b1_sb[:, 0:1], 0.0, OP.add, OP.max,
                ).then_inc(s_dve, 1)
                for n in range(NB):
                    if n + 1 < NB:
                        dve.wait_ge(s_pe, h_ready[n + 1])
                        if n >= 1:
                            dve.wait_ge(s_pe, y_ready[n - 1][NDC - 1])  # h WAR
                        dve.tensor_scalar(
                            h_sb[:, (n + 1) % 2, :], ps_h[(n + 1) % 2][:, :],
                            b1_sb[:, n + 1:n + 2], 0.0, OP.add, OP.max,
                        ).then_inc(s_dve, 1)
                    g, j = divmod(n, NG)
                    for dc in range(NDC):
                        if j == 0:
                            k = tk(1, g, dc)
                            dve.wait_ge(s_slot[k % NSLOT], tslot_done[k])
                        dve.wait_ge(s_act, sig_done[n][dc])
                        if n >= 2:
                            if dc in R_PE:
                                dve.wait_ge(s_pe, racc_done_pe[n - 2][dc])
                            else:
                                dve.wait_ge(s_pool, pool_done[n - 2][dc])
                        slot = tk(1, g, dc) % NSLOT
                        dve.tensor_tensor(
                            p_sb[:, dc, n % 2, :], w_sb[:, dc, n % 2, :],
                            tt_sb[:, slot, j, :], OP.mult,
                        ).then_inc(s_dve, 1)
                for dc in R_PE:
                    dve.wait_ge(s_pe, racc_done_pe[NB - 1][dc])
                    dve.tensor_copy(racc_sb[:, dc, :], r_bank[dc][:, :]).then_inc(s_dve, 1)

            # ================= GPSIMD: weight DMAs (paced) + r accumulation =================
            @block.gpsimd
            def _(pool):
                pool.dma_start(out=ident_sb[:, :], in_=blob_ext[:, OID:OID + 128]).then_inc(s_w, 16)
                pool.dma_start(out=b1_sb[:, :], in_=blob_ext[:, OB1:OB1 + NB]).then_inc(s_w, 16)
                pool.dma_start(out=b2_sb[:, :, :], in_=blob_ext[:, OB2:OB2 + NB * NDC]).then_inc(s_w, 16)

                def wdma(g):
                    a, b = g * NG, (g + 1) * NG
                    pool.dma_start(
                        out=w1_sb[:, a:b, :, :],
                        in_=blob_ext[:, OW1 + a * NDC * INTER:OW1 + b * NDC * INTER],
                    ).then_inc(s_w, 16)
                    pool.dma_start(
                        out=w2_sb[:, a:b, :, :],
                        in_=blob_ext[:, OW2 + a * NDC * INTER:OW2 + b * NDC * INTER],
                    ).then_inc(s_w, 16)

                # pace the first weight groups behind early t chunks so the
                # t stream keeps DMA priority in phase A
                pool.wait_ge(s_pe, pe_groupA_done[5])
                wdma(0)
                pool.wait_ge(s_pe, pe_groupA_done[11])
                wdma(1)
                for n in range(NB):
                    # pace weight group g = n//NG + 2 at each group boundary
                    if n % NG == 0 and n // NG + 2 < NGRP:
                        wdma(n // NG + 2)
                    for dc in R_POOL:
                        pool.wait_ge(s_dve, stt_done[n][dc])
                        if n == 0:
                            pool.tensor_copy(
                                racc_sb[:, dc, :], p_sb[:, dc, 0, :]
                            ).then_inc(s_pool, 1)
                        else:
                            pool.tensor_add(
                                racc_sb[:, dc, :], racc_sb[:, dc, :], p_sb[:, dc, n % 2, :]
                            ).then_inc(s_pool, 1)
                    pool.drain()

    return nc


def _prep(inputs):
    t = inputs["t"]
    W1, b1, g1, beta1, m1, v1 = (
        inputs["W1"], inputs["b1"], inputs["g1"], inputs["beta1"],
        inputs["m1"], inputs["v1"],
    )
    W2, b2, g2, beta2, m2, v2 = (
        inputs["W2"], inputs["b2"], inputs["g2"], inputs["beta2"],
        inputs["m2"], inputs["v2"],
    )
    a1 = g1 / np.sqrt(v1 + EPS)  # (NB, I)
    # t is pre-scaled by 3 on the host (so the device-side gating product
    # 3*w*t is a plain elementwise multiply); compensate in W1.
    W1f = W1 * a1[:, :, None] / 3.0  # (NB, I, D)
    b1f = (b1 - m1) * a1 + beta1  # (NB, I)
    a2 = g2 / np.sqrt(v2 + EPS)  # (NB, D)
    W2f = W2 * a2[:, :, None]  # (NB, D, I)
    b2f = (b2 - m2) * a2 + beta2  # (NB, D)

    bf16 = ml_dtypes.bfloat16
    # w1t[p, n, dc, i] = W1f[n, i, dc*128+p]
    w1t = np.ascontiguousarray(
        W1f.reshape(NB, INTER, NDC, 128).transpose(3, 0, 2, 1)
    ).astype(bf16).reshape(128, W_COLS)
    # w2t[i, n, dc, dd] = W2f[n, dc*128+dd, i]
    w2t = np.ascontiguousarray(
        W2f.reshape(NB, NDC, 128, INTER).transpose(3, 0, 1, 2)
    ).astype(bf16).reshape(128, W_COLS)
    ident = np.eye(128, dtype=bf16)
    # t block: [p, dc, n, b] = 3 * t[b, n, dc*128+p]
    t_bf = (3.0 * t).astype(bf16)  # (B, NB, D)
    t_cols = np.ascontiguousarray(
        t_bf.transpose(2, 1, 0).reshape(NDC, 128, NB, B).transpose(1, 0, 2, 3)
    )  # (128, NDC, NB, B)

    blobs = []
    for c in range(NCORES):
        blob = np.empty((128, CB), bf16)
        blob[:, 0:T_COLS] = t_cols[:, :, :, c * BS:(c + 1) * BS].reshape(128, T_COLS)
        blob[:, OW1:OW1 + W_COLS] = w1t
        blob[:, OW2:OW2 + W_COLS] = w2t
        blob[:, OID:OID + 128] = ident
        blob[:, OB1:OB1 + NB] = b1f.T.astype(bf16)
        blob[:, OB2:OB2 + NB * NDC] = (
            b2f.reshape(NB, NDC, 128).transpose(2, 0, 1).reshape(128, 256).astype(bf16)
        )
        blobs.append(blob)
    return blobs


def kernel(**inputs):
    from concourse.bass_utils import run_bass_kernel_spmd

    blobs = _prep(inputs)

    if "nc" not in _CACHE:
        _CACHE["nc"] = _build_nc()
    nc = _CACHE["nc"]

    in_maps = []
    for c in range(NCORES):
        in_maps.append({"blob": blobs[c]})
    res = run_bass_kernel_spmd(nc, in_maps, core_ids=list(range(NCORES)))
    outs = []
    for c in range(NCORES):
        o = res.results[c]["out"]  # (128, NDC*BS)
        outs.append(o.reshape(128, NDC, BS).transpose(2, 1, 0).reshape(BS, DIM))
    return np.concatenate(outs, axis=0).astype(np.float32)


if __name__ == "__main__":
    rng = np.random.default_rng(0)
    fake = {
        "t": rng.standard_normal((B, NB, DIM), dtype=np.float32),
        "W1": rng.standard_normal((NB, INTER, DIM), dtype=np.float32) * 0.02,
        "b1": rng.standard_normal((NB, INTER), dtype=np.float32) * 0.02,
        "g1": 1 + 0.1 * rng.standard_normal((NB, INTER), dtype=np.float32),
        "beta1": 0.1 * rng.standard_normal((NB, INTER), dtype=np.float32),
        "m1": 0.1 * rng.standard_normal((NB, INTER), dtype=np.float32),
        "v1": rng.uniform(0.5, 1.5, (NB, INTER)).astype(np.float32),
        "W2": rng.standard_normal((NB, DIM, INTER), dtype=np.float32) * 0.02,
        "b2": rng.standard_normal((NB, DIM), dtype=np.float32) * 0.02,
        "g2": 1 + 0.1 * rng.standard_normal((NB, DIM), dtype=np.float32),
        "beta2": 0.1 * rng.standard_normal((NB, DIM), dtype=np.float32),
        "m2": 0.1 * rng.standard_normal((NB, DIM), dtype=np.float32),
        "v2": rng.uniform(0.5, 1.5, (NB, DIM)).astype(np.float32),
    }
    out = kernel(**fake)
    print("kernel ran, out shape", out.shape, out.dtype)
    # quick numpy check of the math
    s = fake["t"].sum(axis=1)
    h = np.einsum('nid,bd->bni', fake["W1"], s) + fake["b1"]
    h = (h - fake["m1"]) / np.sqrt(fake["v1"] + EPS) * fake["g1"] + fake["beta1"]
    h = np.maximum(h, 0)
    y = np.einsum('ndi,bni->bnd', fake["W2"], h) + fake["b2"]
    y = (y - fake["m2"]) / np.sqrt(fake["v2"] + EPS) * fake["g2"] + fake["beta2"]
    w = 1 / (1 + np.exp(-y))
    ref = (w * fake["t"]).sum(axis=1) * 3.0
    err = np.linalg.norm(out - ref) / np.linalg.norm(ref)
    print("rel err vs numpy:", err)



# revision 2
# speedup vs baseline: 2.8768x; 2.8768x over previous
"""Trainium2 Bass kernel for nn_AF_2 (dense per-branch MLP gating), v6.

Math (reference):
    s = t.sum(axis=1)                                  # (B, D)
    h = relu(BN1(einsum('nid,bd->bni', W1, s) + b1))   # (B, NB, I)
    y = BN2(einsum('ndi,bni->bnd', W2, h) + b2)        # (B, NB, D)
    out = (sigmoid(y) * t).sum(axis=1) * 3             # (B, D)

Strategy (v6):
  - 8-way data parallel over B (512 rows/core), zero collectives. Host folds
    the inference-mode BatchNorms into W/b, pre-scales t by 3 (compensated in
    W1), and packs all device inputs into one bf16 DRAM blob.
  - t is streamed twice in 1MB chunks, split evenly across both HWDGE rings
    (SP + Act): each ring carries half of phase A (s-accumulation) and half
    of phase B (gating product). Measured: one ring sustains ~210GB/s, two
    rings ~380GB/s aggregate.
  - s^T accumulated with TensorE identity-matmuls into PSUM (exact f32).
  - Per-branch loop at free-dim 512: G1 (hoisted one branch ahead), BN1+relu
    on VectorE (dual-op tensor_scalar), G2, sigmoid(+bias) on ScalarE,
    product w*t on VectorE (bf16 2x).
  - r accumulation split across engines: dc0/dc1 on TensorE (PSUM
    identity-matmul, exact), dc2 on VectorE (bf16 pairwise tree + f32 group
    accumulator), dc3 on GPSIMD (f32 adds, drain-free, weights front-loaded
    so the Q7 add loop runs without SWDGE descriptor-gen stalls).
  - dc0/dc1 results are staged through the freed product buffer (bitcast to
    f32) so the single out DMA block needs no extra SBUF.
  - build_nc(reps=N) repeats the body N times inside one NEFF with
    iteration-offset semaphore thresholds; slope over N isolates true
    per-execution device time from host/tunnel overhead (see test.py).
"""

import sys

import numpy as np

sys.path.insert(0, "/opt/trn_rl_repo")

import ml_dtypes

B, NB, DIM, R = 4096, 64, 512, 4
INTER = DIM // R  # 128
EPS = 1e-5
NCORES = 8
BS = B // NCORES  # 512 rows per core
NDC = DIM // 128  # 4 d-chunks
NG = 8            # branches per t DMA group
NGRP = NB // NG   # 8 groups
NSLOT = 6         # t ring-buffer slots

# blob column offsets (bf16 blob, [128, CB])
T_COLS = NDC * NB * BS          # 131072
W_COLS = NB * NDC * INTER       # 32768
OW1 = T_COLS
OW2 = OW1 + W_COLS
OID = OW2 + W_COLS
OB1 = OID + 128
OB2 = OB1 + NB
CB = OB2 + NB * NDC             # 197056

DC_PE = (0, 1)   # r-acc on TensorE identity matmuls (PSUM)
DC_TREE = 2      # r-acc on VectorE bf16 pairwise tree
DC_POOL = 3      # r-acc on GPSIMD f32 adds

_CACHE = {}
SIGMA_DUP = 0  # timing probe: issue each sigmoid 1+SIGMA_DUP times


def build_nc(reps=1):
    import concourse.bass as bass
    import concourse.mybir as mybir

    bf16 = mybir.dt.bfloat16
    f32 = mybir.dt.float32
    AF = mybir.ActivationFunctionType
    OP = mybir.AluOpType

    nc = bass.Bass("TRN2", debug=False, target_bir_lowering=False)

    blob_ext = nc.declare_dram_parameter("blob", [128, CB], bf16, isOutput=False)
    out_ext = nc.declare_dram_parameter("out", [128, NDC * BS], f32, isOutput=True)

    def tcol(g, dc):
        return dc * (NB * BS) + g * (NG * BS)

    from contextlib import ExitStack
    ctx = ExitStack()
    with ctx:
        s_w = ctx.enter_context(nc.semaphore("s_w"))
        s_out = ctx.enter_context(nc.semaphore("s_out"))
        s_slot = [ctx.enter_context(nc.semaphore(f"s_slot{i}")) for i in range(NSLOT)]
        s_pe = ctx.enter_context(nc.semaphore("s_pe"))
        s_act = ctx.enter_context(nc.semaphore("s_act"))
        s_dve = ctx.enter_context(nc.semaphore("s_dve"))
        s_pool = ctx.enter_context(nc.semaphore("s_pool"))

        ident_sb = ctx.enter_context(nc.sbuf_tensor("ident_sb", [128, 128], bf16))
        w1_sb = ctx.enter_context(nc.sbuf_tensor("w1_sb", [128, NB, NDC, INTER], bf16))
        w2_sb = ctx.enter_context(nc.sbuf_tensor("w2_sb", [INTER, NB, NDC, 128], bf16))
        b1_sb = ctx.enter_context(nc.sbuf_tensor("b1_sb", [INTER, NB], f32))
        b2_sb = ctx.enter_context(nc.sbuf_tensor("b2_sb", [128, NB, NDC], f32))
        tt_sb = ctx.enter_context(nc.sbuf_tensor("tt_sb", [128, NSLOT, NG, BS], bf16))
        s_sb = ctx.enter_context(nc.sbuf_tensor("s_sb", [128, NDC, BS], bf16))
        h_sb = ctx.enter_context(nc.sbuf_tensor("h_sb", [INTER, 2, BS], bf16))
        w_sb = ctx.enter_context(nc.sbuf_tensor("w_sb", [128, NDC, 2, BS], bf16))
        p01_sb = ctx.enter_context(nc.sbuf_tensor("p01_sb", [128, 2, 2 * BS], bf16))
        p3_sb = ctx.enter_context(nc.sbuf_tensor("p3_sb", [128, 2, BS], bf16))
        gt_sb = ctx.enter_context(nc.sbuf_tensor("gt_sb", [128, 2, BS], bf16))
        ht_sb = ctx.enter_context(nc.sbuf_tensor("ht_sb", [128, 4, BS], bf16))
        # racc holds only dc2 (tree) and dc3 (pool); dc0/dc1 ship straight
        # from their PSUM accumulators.
        racc_sb = ctx.enter_context(nc.sbuf_tensor("racc_sb", [128, 2, BS], f32))

        ps_y = [ctx.enter_context(nc.psum_tensor(f"ps_y{i}", [128, BS], f32)) for i in range(2)]
        ps_s = [ctx.enter_context(nc.psum_tensor(f"ps_s{dc}", [128, BS], f32)) for dc in range(NDC)]
        ps_h = [ctx.enter_context(nc.psum_tensor(f"ps_h{i}", [INTER, BS], f32)) for i in range(2)]
        y_bank = {0: ps_y[0], 1: ps_y[1], 2: ps_s[2], 3: ps_s[3]}
        r_bank = {0: ps_s[0], 1: ps_s[1]}

        # ---------- per-iteration static schedule ----------
        n_tdma = 2 * NGRP * NDC  # 64 t-chunk DMAs (32 phase A + 32 phase B)
        tslot_done = [16 * (k // NSLOT + 1) for k in range(n_tdma)]
        slot_counts = [sum(1 for k in range(n_tdma) if k % NSLOT == i) for i in range(NSLOT)]
        D_SLOT = [16 * c for c in slot_counts]

        def tk(phase, g, dc):
            return phase * NGRP * NDC + g * NDC + dc

        # ----- PE schedule -----
        pe_A_done = [None] * (NGRP * NDC)
        h_ready = [None] * NB
        y_ready = [[None] * NDC for _ in range(NB)]
        racc_done_pe = [[None] * NDC for _ in range(NB)]
        pe_i = 0
        for k in range(NGRP * NDC):
            pe_i += 1
            pe_A_done[k] = pe_i
        pe_i += 1
        h_ready[0] = pe_i
        for n in range(NB):
            if n + 1 < NB:
                pe_i += 1
                h_ready[n + 1] = pe_i
            for dc in range(NDC):
                pe_i += 1
                y_ready[n][dc] = pe_i
            if n > 0:
                for dc in DC_PE:
                    pe_i += 1
                    racc_done_pe[n - 1][dc] = pe_i
        for dc in DC_PE:
            pe_i += 1
            racc_done_pe[NB - 1][dc] = pe_i
        D_PE = pe_i

        # ----- Act schedule -----
        sig_done = [[None] * NDC for _ in range(NB)]
        act_i = 0
        for n in range(NB):
            for dc in range(NDC):
                act_i += 1 + SIGMA_DUP
                sig_done[n][dc] = act_i
        D_ACT = act_i

        # ----- DVE schedule -----
        scopy_done = [None] * NDC
        relu_done = [None] * NB
        stt_done = [[None] * NDC for _ in range(NB)]
        tree_acc_done = [None] * NGRP
        dve_i = 0
        for dc in range(NDC):
            dve_i += 1
            scopy_done[dc] = dve_i
        dve_i += 1
        relu_done[0] = dve_i
        for n in range(NB):
            if n + 1 < NB:
                dve_i += 1
                relu_done[n + 1] = dve_i
            for dc in range(NDC):
                dve_i += 1
                stt_done[n][dc] = dve_i
                if dc == DC_TREE and n % 2 == 1:
                    dve_i += 1
            if n % NG == NG - 1:
                dve_i += 4
                tree_acc_done[n // NG] = dve_i
        rcopy_done = {}
        for dc in DC_PE:
            dve_i += 1
            rcopy_done[dc] = dve_i
        D_DVE = dve_i

        # ----- Pool schedule -----
        pool_done = [None] * NB
        pool_i = 0
        for n in range(NB):
            pool_i += 1
            pool_done[n] = pool_i
        D_POOL = pool_i

        D_W = 16 * (3 + 2 * NGRP)
        D_OUT = 48  # three out DMAs x16

        def wg_done(g):
            return 16 * (3 + 2 * (g + 1))

        # generic chunk-issue with slot-reuse wait (previous occupant = k-NSLOT)
        def issue_chunk(eng, k, bpe, bdve, bslot):
            slot = k % NSLOT
            pk = k - NSLOT
            if pk >= 0:
                if pk < NGRP * NDC:
                    eng.wait_ge(s_pe, bpe + pe_A_done[pk])
                else:
                    pg, pdc = divmod(pk - NGRP * NDC, NDC)
                    eng.wait_ge(s_dve, bdve + stt_done[pg * NG + NG - 1][pdc])
            if k < NGRP * NDC:
                g, dc = divmod(k, NDC)
            else:
                g, dc = divmod(k - NGRP * NDC, NDC)
            c0 = tcol(g, dc)
            eng.dma_start(
                out=tt_sb[:, slot, :, :],
                in_=blob_ext[:, c0:c0 + NG * BS],
            ).then_inc(s_slot[slot], 16)

        with nc.Block() as block:

            # ===== SP ring: even phase-A chunks; phase-B dc0/dc2; out =====
            @block.sync
            def _(sp):
                for it in range(reps):
                    bpe, bdve, bpool, bact = (
                        it * D_PE, it * D_DVE, it * D_POOL, it * D_ACT)
                    bslot = [it * d for d in D_SLOT]
                    for k in range(NGRP * NDC):
                        if k % 2 == 0:
                            issue_chunk(sp, k, bpe, bdve, bslot)
                    for g in range(NGRP):
                        for dc in (0, 2):
                            issue_chunk(sp, tk(1, g, dc), bpe, bdve, bslot)
                    for dc in DC_PE:
                        sp.wait_ge(s_dve, bdve + rcopy_done[dc])
                    sp.dma_start(out=out_ext[:, 0:BS],
                                 in_=p01_sb[:, 0, :].bitcast(f32)).then_inc(s_out, 16)
                    sp.dma_start(out=out_ext[:, BS:2 * BS],
                                 in_=p01_sb[:, 1, :].bitcast(f32)).then_inc(s_out, 16)
                    sp.wait_ge(s_dve, bdve + tree_acc_done[NGRP - 1])
                    sp.wait_ge(s_pool, bpool + pool_done[NB - 1])
                    sp.dma_start(out=out_ext[:, 2 * BS:4 * BS], in_=racc_sb[:, :, :]).then_inc(s_out, 16)
                    sp.wait_ge(s_out, it * D_OUT + 48)

            # ===== PE: s-sum identities, G1, G2, r-acc dc0/1 =====
            @block.tensor
            def _(pe):
                for it in range(reps):
                    bpe, bact, bdve, bw = it * D_PE, it * D_ACT, it * D_DVE, it * D_W
                    bslot = [it * d for d in D_SLOT]
                    if it > 0:
                        pe.wait_ge(s_act, it * D_ACT)  # y banks (ps_s2/3) free
                        pe.wait_ge(s_dve, it * D_DVE)  # r banks copied out
                    pe.wait_ge(s_w, bw + 16 * 3)
                    for g in range(NGRP):
                        for dc in range(NDC):
                            k = tk(0, g, dc)
                            slot = k % NSLOT
                            pe.wait_ge(s_slot[slot], bslot[slot] + tslot_done[k])
                            for j in range(NG):
                                mm = pe.matmul(
                                    ps_s[dc][:, :], lhsT=ident_sb[:, :],
                                    rhs=tt_sb[:, slot, j, :],
                                    start=(g == 0 and j == 0), stop=(g == NGRP - 1 and j == NG - 1),
                                )
                            mm.then_inc(s_pe, 1)
                    pe.wait_ge(s_dve, bdve + scopy_done[NDC - 1])
                    pe.wait_ge(s_w, bw + wg_done(0))
                    for dc in range(NDC):
                        mm = pe.matmul(
                            ps_h[0][:, :], lhsT=w1_sb[:, 0, dc, :], rhs=s_sb[:, dc, :],
                            start=(dc == 0), stop=(dc == NDC - 1),
                        )
                    mm.then_inc(s_pe, 1)
                    for n in range(NB):
                        if n + 1 < NB:
                            if (n + 1) % NG == 0:
                                pe.wait_ge(s_w, bw + wg_done((n + 1) // NG))
                            for dc in range(NDC):
                                mm = pe.matmul(
                                    ps_h[(n + 1) % 2][:, :], lhsT=w1_sb[:, n + 1, dc, :],
                                    rhs=s_sb[:, dc, :],
                                    start=(dc == 0), stop=(dc == NDC - 1),
                                )
                            mm.then_inc(s_pe, 1)
                        pe.wait_ge(s_dve, bdve + relu_done[n])
                        for dc in range(NDC):
                            if n > 0:
                                pe.wait_ge(s_act, bact + sig_done[n - 1][dc])
                            elif dc >= 2:
                                pe.wait_ge(s_dve, bdve + scopy_done[dc])
                            pe.matmul(
                                y_bank[dc][:, :],
                                lhsT=w2_sb[:, n, dc, :], rhs=h_sb[:, n % 2, :],
                                start=True, stop=True,
                            ).then_inc(s_pe, 1)
                        if n > 0:
                            for dc in DC_PE:
                                pe.wait_ge(s_dve, bdve + stt_done[n - 1][dc])
                                pe.matmul(
                                    r_bank[dc][:, :], lhsT=ident_sb[:, :],
                                    rhs=p01_sb[:, dc, ((n - 1) % 2) * BS:((n - 1) % 2) * BS + BS],
                                    start=(n - 1 == 0), stop=False,
                                ).then_inc(s_pe, 1)
                    for dc in DC_PE:
                        pe.wait_ge(s_dve, bdve + stt_done[NB - 1][dc])
                        pe.matmul(
                            r_bank[dc][:, :], lhsT=ident_sb[:, :],
                            rhs=p01_sb[:, dc, ((NB - 1) % 2) * BS:((NB - 1) % 2) * BS + BS],
                            start=False, stop=True,
                        ).then_inc(s_pe, 1)

            # ===== Act ring: odd phase-A chunks; phase-B dc1/dc3; sigmoids =====
            @block.scalar
            def _(act):
                for it in range(reps):
                    bpe, bact, bdve, bw = it * D_PE, it * D_ACT, it * D_DVE, it * D_W
                    bslot = [it * d for d in D_SLOT]
                    if it > 0:
                        act.wait_ge(s_dve, it * D_DVE)
                    for k in range(NGRP * NDC):
                        if k % 2 == 1:
                            issue_chunk(act, k, bpe, bdve, bslot)
                    act.wait_ge(s_w, bw + 16 * 3)
                    for n in range(NB):
                        if n == 0:
                            # dc1(g0), dc3(g0), dc1(g1): all phase-A slot deps
                            issue_chunk(act, tk(1, 0, 1), bpe, bdve, bslot)
                            issue_chunk(act, tk(1, 0, 3), bpe, bdve, bslot)
                            issue_chunk(act, tk(1, 1, 1), bpe, bdve, bslot)
                        elif n % NG == 0:
                            g = n // NG
                            issue_chunk(act, tk(1, g, 3), bpe, bdve, bslot)
                            if g + 1 < NGRP:
                                issue_chunk(act, tk(1, g + 1, 1), bpe, bdve, bslot)
                        if n >= 2:
                            act.wait_ge(s_dve, bdve + stt_done[n - 2][NDC - 1])
                        for dc in range(NDC):
                            act.wait_ge(s_pe, bpe + y_ready[n][dc])
                            for _rep in range(1 + SIGMA_DUP):
                                act.activation(
                                    w_sb[:, dc, n % 2, :], y_bank[dc][:, :], AF.Sigmoid,
                                    bias=b2_sb[:, n, dc:dc + 1], scale=1.0,
                                ).then_inc(s_act, 1)

            # ===== DVE: s-copies, bn1+relu, products, dc2 tree =====
            @block.vector
            def _(dve):
                for it in range(reps):
                    bpe, bact, bdve, bw, bpool, bout = (
                        it * D_PE, it * D_ACT, it * D_DVE, it * D_W, it * D_POOL,
                        it * D_OUT)
                    bslot = [it * d for d in D_SLOT]
                    dve.wait_ge(s_pe, bpe + pe_A_done[NGRP * NDC - 1])
                    for dc in range(NDC):
                        dve.tensor_copy(s_sb[:, dc, :], ps_s[dc][:, :]).then_inc(s_dve, 1)
                    dve.wait_ge(s_w, bw + 16 * 3)
                    dve.wait_ge(s_pe, bpe + h_ready[0])
                    dve.tensor_scalar(
                        h_sb[:, 0, :], ps_h[0][:, :], b1_sb[:, 0:1], 0.0, OP.add, OP.max,
                    ).then_inc(s_dve, 1)
                    hti = 0
                    for n in range(NB):
                        if n + 1 < NB:
                            dve.wait_ge(s_pe, bpe + h_ready[n + 1])
                            if n >= 1:
                                dve.wait_ge(s_pe, bpe + y_ready[n - 1][NDC - 1])
                            dve.tensor_scalar(
                                h_sb[:, (n + 1) % 2, :], ps_h[(n + 1) % 2][:, :],
                                b1_sb[:, n + 1:n + 2], 0.0, OP.add, OP.max,
                            ).then_inc(s_dve, 1)
                        g, j = divmod(n, NG)
                        for dc in range(NDC):
                            if j == 0:
                                k = tk(1, g, dc)
                                dve.wait_ge(s_slot[k % NSLOT], bslot[k % NSLOT] + tslot_done[k])
                            dve.wait_ge(s_act, bact + sig_done[n][dc])
                            slot = tk(1, g, dc) % NSLOT
                            tsrc = tt_sb[:, slot, j, :]
                            wsrc = w_sb[:, dc, n % 2, :]
                            if dc in DC_PE:
                                if n >= 2:
                                    dve.wait_ge(s_pe, bpe + racc_done_pe[n - 2][dc])
                                dve.tensor_tensor(
                                    p01_sb[:, dc, (n % 2) * BS:(n % 2) * BS + BS], wsrc, tsrc, OP.mult,
                                ).then_inc(s_dve, 1)
                            elif dc == DC_POOL:
                                if n >= 2:
                                    dve.wait_ge(s_pool, bpool + pool_done[n - 2])
                                dve.tensor_tensor(
                                    p3_sb[:, n % 2, :], wsrc, tsrc, OP.mult,
                                ).then_inc(s_dve, 1)
                            else:  # DC_TREE
                                dve.tensor_tensor(
                                    gt_sb[:, n % 2, :], wsrc, tsrc, OP.mult,
                                ).then_inc(s_dve, 1)
                                if n % 2 == 1:
                                    dve.tensor_tensor(
                                        ht_sb[:, hti, :], gt_sb[:, 0, :], gt_sb[:, 1, :],
                                        OP.add,
                                    ).then_inc(s_dve, 1)
                                    hti = (hti + 1) % 4
                        if j == NG - 1:
                            dve.tensor_tensor(
                                gt_sb[:, 0, :], ht_sb[:, 0, :], ht_sb[:, 1, :], OP.add,
                            ).then_inc(s_dve, 1)
                            dve.tensor_tensor(
                                gt_sb[:, 1, :], ht_sb[:, 2, :], ht_sb[:, 3, :], OP.add,
                            ).then_inc(s_dve, 1)
                            dve.tensor_tensor(
                                ht_sb[:, 0, :], gt_sb[:, 0, :], gt_sb[:, 1, :], OP.add,
                            ).then_inc(s_dve, 1)
                            if g == 0:
                                if it > 0:
                                    dve.wait_ge(s_out, (it - 1) * D_OUT + 48)
                                dve.tensor_copy(
                                    racc_sb[:, 0, :], ht_sb[:, 0, :],
                                ).then_inc(s_dve, 1)
                            else:
                                dve.tensor_tensor(
                                    racc_sb[:, 0, :], racc_sb[:, 0, :],
                                    ht_sb[:, 0, :], OP.add,
                                ).then_inc(s_dve, 1)
                    for dc in DC_PE:
                        dve.wait_ge(s_pe, bpe + racc_done_pe[NB - 1][dc])
                        dve.tensor_copy(
                            p01_sb[:, dc, :].bitcast(f32), r_bank[dc][:, :],
                        ).then_inc(s_dve, 1)

            # ===== Pool: weights (SWDGE) + r-acc dc3 =====
            @block.gpsimd
            def _(pool):
                for it in range(reps):
                    bpe, bact, bdve, bw, bout = (
                        it * D_PE, it * D_ACT, it * D_DVE, it * D_W, it * D_OUT)
                    if it > 0:
                        pool.wait_ge(s_pe, it * D_PE)
                        pool.wait_ge(s_act, it * D_ACT)
                        pool.wait_ge(s_dve, it * D_DVE)
                    pool.dma_start(out=ident_sb[:, :], in_=blob_ext[:, OID:OID + 128]).then_inc(s_w, 16)
                    pool.dma_start(out=b1_sb[:, :], in_=blob_ext[:, OB1:OB1 + NB]).then_inc(s_w, 16)
                    pool.dma_start(out=b2_sb[:, :, :], in_=blob_ext[:, OB2:OB2 + NB * NDC]).then_inc(s_w, 16)

                    def wdma(g):
                        a, b = g * NG, (g + 1) * NG
                        pool.dma_start(
                            out=w1_sb[:, a:b, :, :],
                            in_=blob_ext[:, OW1 + a * NDC * INTER:OW1 + b * NDC * INTER],
                        ).then_inc(s_w, 16)
                        pool.dma_start(
                            out=w2_sb[:, a:b, :, :],
                            in_=blob_ext[:, OW2 + a * NDC * INTER:OW2 + b * NDC * INTER],
                        ).then_inc(s_w, 16)

                    pool.wait_ge(s_pe, bpe + pe_A_done[5])
                    wdma(0)
                    pool.wait_ge(s_pe, bpe + pe_A_done[11])
                    wdma(1)
                    # queue the remaining weight groups up front: desc-gen all
                    # at once so the add loop below runs without SWDGE stalls
                    pool.wait_ge(s_pe, bpe + pe_A_done[NGRP * NDC - 1])
                    for g in range(2, NGRP):
                        wdma(g)
                    for n in range(NB):
                        pool.wait_ge(s_dve, bdve + stt_done[n][DC_POOL])
                        if n == 0:
                            if it > 0:
                                pool.wait_ge(s_out, (it - 1) * D_OUT + 48)
                            pool.tensor_copy(
                                racc_sb[:, 1, :], p3_sb[:, 0, :]
                            ).then_inc(s_pool, 1)
                        else:
                            pool.tensor_add(
                                racc_sb[:, 1, :], racc_sb[:, 1, :],
                                p3_sb[:, n % 2, :]
                            ).then_inc(s_pool, 1)

    return nc


def _prep(inputs):
    t = inputs["t"]
    W1, b1, g1, beta1, m1, v1 = (
        inputs["W1"], inputs["b1"], inputs["g1"], inputs["beta1"],
        inputs["m1"], inputs["v1"],
    )
    W2, b2, g2, beta2, m2, v2 = (
        inputs["W2"], inputs["b2"], inputs["g2"], inputs["beta2"],
        inputs["m2"], inputs["v2"],
    )
    a1 = g1 / np.sqrt(v1 + EPS)  # (NB, I)
    W1f = W1 * a1[:, :, None] / 3.0  # (NB, I, D); t pre-scaled by 3
    b1f = (b1 - m1) * a1 + beta1  # (NB, I)
    a2 = g2 / np.sqrt(v2 + EPS)  # (NB, D)
    W2f = W2 * a2[:, :, None]  # (NB, D, I)
    b2f = (b2 - m2) * a2 + beta2  # (NB, D)

    bf16 = ml_dtypes.bfloat16
    w1t = np.ascontiguousarray(
        W1f.reshape(NB, INTER, NDC, 128).transpose(3, 0, 2, 1)
    ).astype(bf16).reshape(128, W_COLS)
    w2t = np.ascontiguousarray(
        W2f.reshape(NB, NDC, 128, INTER).transpose(3, 0, 1, 2)
    ).astype(bf16).reshape(128, W_COLS)
    ident = np.eye(128, dtype=bf16)
    t_bf = (3.0 * t).astype(bf16)  # (B, NB, D)
    t_cols = np.ascontiguousarray(
        t_bf.transpose(2, 1, 0).reshape(NDC, 128, NB, B).transpose(1, 0, 2, 3)
    )  # (128, NDC, NB, B)

    blobs = []
    for c in range(NCORES):
        blob = np.empty((128, CB), bf16)
        blob[:, 0:T_COLS] = t_cols[:, :, :, c * BS:(c + 1) * BS].reshape(128, T_COLS)
        blob[:, OW1:OW1 + W_COLS] = w1t
        blob[:, OW2:OW2 + W_COLS] = w2t
        blob[:, OID:OID + 128] = ident
        blob[:, OB1:OB1 + NB] = b1f.T.astype(bf16)
        blob[:, OB2:OB2 + NB * NDC] = (
            b2f.reshape(NB, NDC, 128).transpose(2, 0, 1).reshape(128, 256).astype(bf16)
        )
        blobs.append(blob)
    return blobs


def kernel(**inputs):
    from concourse.bass_utils import run_bass_kernel_spmd

    blobs = _prep(inputs)

    if "nc" not in _CACHE:
        _CACHE["nc"] = build_nc(1)
    nc = _CACHE["nc"]

    in_maps = []
    for c in range(NCORES):
        in_maps.append({"blob": blobs[c]})
    res = run_bass_kernel_spmd(nc, in_maps, core_ids=list(range(NCORES)))
    outs = []
    for c in range(NCORES):
        o = res.results[c]["out"]  # (128, NDC*BS)
        outs.append(o.reshape(128, NDC, BS).transpose(2, 1, 0).reshape(BS, DIM))
    return np.concatenate(outs, axis=0).astype(np.float32)


# compat aliases so shared tooling can introspect
_build_nc = build_nc
build_nc_reps = build_nc
